# revision 18
# baseline (speedup 1.0000x reference)
"""Bass/Tile kernel for nn_CrossAttention_RoPE on TRN2, data-parallel over batch.

v2: software-pipelined per-L-tile design.
 - fused roped q (single contraction) halves logits matmuls + q transposes
 - per-head exp with accum_out -> softmax denominators for free
 - sm/||q|| folded into one alpha multiply on the q path
 - softmax divide folded into the PV PSUM->SBUF copy via a PE-expanded
   reciprocal tile
 - 3-tile stage skew in emission order keeps the PE queue saturated
"""
import numpy as np
import concourse.bass as bass
import concourse.mybir as mybir
import concourse.tile as tile
from concourse import bacc
from concourse.bass_utils import run_bass_kernel_spmd
from concourse.masks import make_identity

F32 = mybir.dt.float32
BF16 = mybir.dt.bfloat16

# ---- problem constants ----
B, L, C, Lk, H, D = 8, 1704, 1024, 144, 16, 64
LP = 1792           # L padded to 14*128
NLT = LP // 128     # 14 L tiles
MAX_SCALE_MUL = float(np.log(100.0))


def precompute_freqs_cis(dim, patch_nums, theta=10000.0):
    freqs = 1.0 / theta ** (np.arange(0, dim, 4)[: dim // 4].astype(np.float32) / dim)
    tx, ty = [], []
    grid = 32.0
    for p in patch_nums:
        ix, iy = np.meshgrid(np.arange(p), np.arange(p), indexing="ij")
        tx.append(ix.flatten().astype(np.float32) / p * grid)
        ty.append(iy.flatten().astype(np.float32) / p * grid)
    tx = np.concatenate(tx)
    ty = np.concatenate(ty)
    ang = np.concatenate([np.outer(tx, freqs), np.outer(ty, freqs)], axis=1).astype(np.float32)
    return np.stack([np.cos(ang), np.sin(ang)], axis=-1)  # [Lx, dim//2, 2]


def rope_tables(fc, n_rows):
    """fc: [n, 32, 2] -> C [n_rows, 64] (cos dup), NS [n_rows, 32] (-sin), PS [n_rows, 32] (+sin)."""
    n = fc.shape[0]
    Ct = np.zeros((n_rows, 64), np.float32)
    NS = np.zeros((n_rows, 32), np.float32)
    PS = np.zeros((n_rows, 32), np.float32)
    cos, sin = fc[..., 0], fc[..., 1]
    Ct[:n, 0::2] = cos
    Ct[:n, 1::2] = cos
    NS[:n] = -sin
    PS[:n] = sin
    return Ct, NS, PS


def host_prep(inputs):
    import ml_dtypes
    bf = ml_dtypes.bfloat16
    x = np.asarray(inputs["x"], np.float32)
    y = np.asarray(inputs["y"], np.float32)
    fc = np.asarray(inputs["freqs_cis"], np.float32)
    ab = np.asarray(inputs["attn_bias"], np.float32).reshape(L, Lk)
    Wq = np.asarray(inputs["Wq"], np.float32)
    Wkv = np.asarray(inputs["Wkv"], np.float32)
    Wproj = np.asarray(inputs["Wproj"], np.float32)
    sm = np.exp(np.minimum(np.asarray(inputs["scale_mul"], np.float32), MAX_SCALE_MUL)).reshape(H)

    Cq, NSq, PSq = rope_tables(fc, LP)
    qtab = np.zeros((LP, 128), np.float32)
    qtab[:, 0:64] = Cq
    qtab[:, 64:96] = NSq
    qtab[:, 96:128] = PSq

    fck = precompute_freqs_cis(D, [12])
    Ck, NSk, PSk = rope_tables(fck, Lk)

    bias2d = np.zeros((LP, Lk), np.float32)
    bias2d[:L] = ab
    bias2 = np.tile(bias2d, (1, 2)).astype(bf)  # [LP, 288]

    sel = np.zeros((16, 8 * 128), np.float32)
    for p in range(8):
        for m in range(128):
            sel[2 * p + (m >= 64), 128 * p + m] = 1.0

    shared = {
        "wqT": np.ascontiguousarray(Wq.T).astype(bf),
        "wkT": np.ascontiguousarray(Wkv[:C].T).astype(bf),
        "wvT": np.ascontiguousarray(Wkv[C:].T).astype(bf),
        "wpT": np.ascontiguousarray(Wproj.T).astype(bf),
        "smv": sm.astype(np.float32),
        "qtab": qtab.astype(bf),
        "ck": Ck.astype(bf), "nsk": NSk.astype(bf), "psk": PSk.astype(bf),
        "bias2": bias2,
        "sel": sel.astype(bf),
    }
    xTp = np.zeros((B, C, LP), np.float32)
    xTp[:, :, :L] = x.transpose(0, 2, 1)
    in_maps = []
    for b in range(B):
        m = dict(shared)
        m["xT"] = np.ascontiguousarray(xTp[b]).astype(bf)
        m["yT"] = np.ascontiguousarray(y[b].T).astype(bf)
        in_maps.append(m)
    return in_maps


def build():
    nc = bacc.Bacc("TRN2", target_bir_lowering=False, debug=False, num_devices=8)
    dram = {}
    for name, shape, dt in [
        ("xT", [C, LP], BF16), ("yT", [C, Lk], BF16),
        ("wqT", [C, C], BF16), ("wkT", [C, C], BF16),
        ("wvT", [C, C], BF16), ("wpT", [C, C], BF16),
        ("smv", [H], F32),
        ("qtab", [LP, 128], BF16),
        ("ck", [Lk, 64], BF16), ("nsk", [Lk, 32], BF16), ("psk", [Lk, 32], BF16),
        ("bias2", [LP, 2 * Lk], BF16),
        ("sel", [16, 8 * 128], BF16),
    ]:
        dram[name] = nc.dram_tensor(name, shape, dt, kind="ExternalInput").ap()
    out_d = nc.dram_tensor("out", [LP, C], F32, kind="ExternalOutput").ap()

    with tile.TileContext(nc) as tc:
        kernel_body(tc, dram, out_d)
    nc.compile()
    return nc


def kernel_body(tc, dram, out_d):
    nc = tc.nc
    AX = mybir.AxisListType.X
    AF = mybir.ActivationFunctionType

    from contextlib import ExitStack
    ctx = ExitStack()
    # SBUF pools
    wts = ctx.enter_context(tc.tile_pool(name="wts", bufs=32))
    const = ctx.enter_context(tc.tile_pool(name="const", bufs=1))
    kvp = ctx.enter_context(tc.tile_pool(name="kvp", bufs=1))
    xts = ctx.enter_context(tc.tile_pool(name="xts", bufs=3))
    qtabs = ctx.enter_context(tc.tile_pool(name="qtabs", bufs=3))
    biasp = ctx.enter_context(tc.tile_pool(name="biasp", bufs=3))
    sqp = ctx.enter_context(tc.tile_pool(name="sqp", bufs=2))
    smalls = ctx.enter_context(tc.tile_pool(name="smalls", bufs=2))
    qwork = ctx.enter_context(tc.tile_pool(name="qwork", bufs=2))
    qTp = ctx.enter_context(tc.tile_pool(name="qTp", bufs=2))
    atp = ctx.enter_context(tc.tile_pool(name="atp", bufs=2))
    aTp = ctx.enter_context(tc.tile_pool(name="aTp", bufs=2))
    oupp = ctx.enter_context(tc.tile_pool(name="oupp", bufs=2))
    outp = ctx.enter_context(tc.tile_pool(name="outp", bufs=2))
    # PSUM pools: 4 + 2 + 2 = 8 banks
    pmm = ctx.enter_context(tc.tile_pool(name="pmm", bufs=4, space="PSUM"))
    plg = ctx.enter_context(tc.tile_pool(name="plg", bufs=2, space="PSUM"))
    ptp = ctx.enter_context(tc.tile_pool(name="ptp", bufs=2, space="PSUM"))

    def mm(out, lhsT, rhs, start, stop, **kw):
        nc.tensor.matmul(out, lhsT, rhs, start=start, stop=stop, **kw)

    def tr(out, in_, idt):
        nc.tensor.matmul(out, in_, idt, is_transpose=True, skip_group_check=True,
                         tile_position=(in_.base_partition(), out.base_partition()))

    # ---------------- constants / setup ----------------
    ident = const.tile([128, 128], BF16)
    make_identity(nc, ident[:])
    eps = const.tile([128, 1], F32)
    nc.vector.memset(eps[:], 1e-20)
    sm_r = const.tile([128, H], F32)
    nc.sync.dma_start(sm_r[:], dram["smv"].unsqueeze(0).to_broadcast((128, H)))
    sel_sb = const.tile([16, 8 * 128], BF16)
    nc.sync.dma_start(sel_sb[:], dram["sel"])

    def load_w(name):
        ts_ = []
        for kc in range(8):
            t = wts.tile([128, C], BF16, tag="w")
            nc.sync.dma_start(t[:], dram[name][kc * 128:(kc + 1) * 128, :])
            ts_.append(t)
        return ts_

    # DMA order matters at startup: yt+wk feed the K projection (first PE
    # work); wq feeds Qproj(0); wv/wp only needed later.
    yt = []
    for kc in range(8):
        t = kvp.tile([128, Lk], BF16, tag=f"yt{kc}")
        nc.sync.dma_start(t[:], dram["yT"][kc * 128:(kc + 1) * 128, :])
        yt.append(t)
    wk = load_w("wkT")
    wq = load_w("wqT")
    wv = load_w("wvT")
    wp = load_w("wpT")

    # k rope tables
    ckt = const.tile([128, 64], BF16)
    nskt = const.tile([128, 32], BF16)
    pskt = const.tile([128, 32], BF16)
    nc.sync.dma_start(ckt[:], dram["ck"][0:128, :])
    nc.sync.dma_start(nskt[:], dram["nsk"][0:128, :])
    nc.sync.dma_start(pskt[:], dram["psk"][0:128, :])
    ckt2 = const.tile([16, 64], BF16)
    nskt2 = const.tile([16, 32], BF16)
    pskt2 = const.tile([16, 32], BF16)
    nc.sync.dma_start(ckt2[:], dram["ck"][128:Lk, :])
    nc.sync.dma_start(nskt2[:], dram["nsk"][128:Lk, :])
    nc.sync.dma_start(pskt2[:], dram["psk"][128:Lk, :])

    # ---- K/V projections: natural [Lk(128+16), C] ----
    def kv_proj(wtiles, label):
        mats = []
        for mt, msz in [(0, 128), (1, 16)]:
            sb = kvp.tile([msz, C], BF16, tag=f"{label}{mt}")
            for nc2 in range(2):
                ps = pmm.tile([msz, 512], F32, tag="mm")
                for kc in range(8):
                    mm(ps[:], yt[kc][:, mt * 128: mt * 128 + msz],
                       wtiles[kc][:, nc2 * 512:(nc2 + 1) * 512],
                       (kc == 0), (kc == 7))
                nc.scalar.copy(sb[:, nc2 * 512:(nc2 + 1) * 512], ps[:])
            mats.append(sb)
        return mats

    k_nat = kv_proj(wk, "knat")

    def k_norm_rope(src, msz, ct, nst, pst):
        """src [msz, C] -> roped unit-norm k [msz, C] bf16."""
        sq = sqp.tile([msz, C], F32, tag="ksq")
        nc.scalar.activation(sq[:], src[:], AF.Square)
        s16 = smalls.tile([msz, H], F32, tag="ks16")
        nc.vector.reduce_sum(s16[:], sq[:].rearrange("p (h d) -> p h d", d=D), axis=AX)
        rt = smalls.tile([msz, H], F32, tag="krt")
        nc.scalar.activation(rt[:], s16[:], AF.Sqrt, bias=eps[:msz, :])
        rq = smalls.tile([msz, H], F32, tag="krq")
        nc.vector.reciprocal(rq[:], rt[:])
        hat = qwork.tile([msz, C], BF16, tag="khat")
        nc.vector.tensor_mul(
            hat[:].rearrange("p (h d) -> p h d", d=D),
            src[:].rearrange("p (h d) -> p h d", d=D),
            rq[:].unsqueeze(2).to_broadcast((msz, H, D)))
        ka = qwork.tile([msz, C], BF16, tag="kka")
        nc.vector.tensor_mul(
            ka[:].rearrange("p (h d) -> p h d", d=D),
            hat[:].rearrange("p (h d) -> p h d", d=D),
            ct[:msz, :].unsqueeze(1).to_broadcast((msz, H, D)))
        kb = qwork.tile([msz, C], BF16, tag="kkb")
        hat4 = hat[:].rearrange("p (h j t) -> p h j t", j=32, t=2)
        kb4 = kb[:].rearrange("p (h j t) -> p h j t", j=32, t=2)
        nc.vector.tensor_mul(
            kb4[:, :, :, 0:1].squeeze(3),
            hat4[:, :, :, 1:2].squeeze(3),
            nst[:msz, :].unsqueeze(1).to_broadcast((msz, H, 32)))
        nc.vector.tensor_mul(
            kb4[:, :, :, 1:2].squeeze(3),
            hat4[:, :, :, 0:1].squeeze(3),
            pst[:msz, :].unsqueeze(1).to_broadcast((msz, H, 32)))
        kp = kvp.tile([msz, C], BF16, tag=f"kp{msz}")
        nc.vector.tensor_add(kp[:], ka[:], kb[:])
        return kp

    kp_m = k_norm_rope(k_nat[0], 128, ckt, nskt, pskt)
    kp_t = k_norm_rope(k_nat[1], 16, ckt2, nskt2, pskt2)

    # block-diagonal kT per pair: [128 (2h x 64d), 288 (2 x 144k)] so one
    # matmul with the full 128-row qT pair computes both heads' logits.
    kT = []
    for t in range(8):
        ps = ptp.tile([128, Lk], BF16, tag="tp")
        for hh in range(2):
            h = 2 * t + hh
            tr(ps[64 * hh:64 * hh + 64, 0:128], kp_m[:, h * D:(h + 1) * D], ident[:])
            tr(ps[64 * hh:64 * hh + 64, 128:Lk], kp_t[:, h * D:(h + 1) * D],
               ident[:16, :16])
        sb = kvp.tile([128, 2 * Lk], BF16, tag=f"kT{t}")
        nc.vector.memset(sb[:], 0)
        nc.vector.tensor_copy(sb[0:64, 0:Lk], ps[0:64, :])
        nc.vector.tensor_copy(sb[64:128, Lk:2 * Lk], ps[64:128, :])
        kT.append(sb)

    v_nat = kv_proj(wv, "vnat")
    v_m, v_t = v_nat[0], v_nat[1]
    # block-diag packed tail-V at row offset 32*(p%4) to match tailT slices
    vtz = kvp.tile([128, 8 * 128], BF16, tag="vtz")
    nc.vector.memset(vtz[:], 0)
    for p in range(8):
        r = 32 * (p % 4)
        nc.sync.dma_start(vtz[r:r + 16, 128 * p:128 * p + 64],
                          v_t[:, 64 * (2 * p):64 * (2 * p) + 64])
        nc.sync.dma_start(vtz[r + 16:r + 32, 128 * p + 64:128 * p + 128],
                          v_t[:, 64 * (2 * p + 1):64 * (2 * p + 1) + 64])

    # ---------------- per-tile stage functions ----------------
    xt_t = [None] * NLT
    qtab_t = [None] * NLT
    bias_t = [None] * NLT
    psq_t = [None] * NLT       # Q-proj psum pair
    sq_t = [None] * NLT
    alpha_t = [None] * NLT
    qr_t = [None] * NLT
    qT_t = [None] * NLT
    lg_t = [None] * NLT        # logits psum pair tiles (list of 8)
    at_t = [None] * NLT
    s_t = [None] * NLT
    rec_t = [None] * NLT
    recT_t = [None] * NLT
    tails_t = [None] * NLT
    aT_t = [None] * NLT
    tailT_t = [None] * NLT
    mult_t = [None] * NLT
    pso_t = [None] * NLT
    oupT_t = [None] * NLT

    def dma_in(i):
        xt = xts.tile([128, 8 * 128], BF16, tag="xt")
        nc.sync.dma_start(
            xt[:].rearrange("p (k l) -> p k l", l=128),
            dram["xT"].rearrange("(k p) l -> p k l", p=128)[:, :, i * 128:(i + 1) * 128])
        xt_t[i] = xt
        qt = qtabs.tile([128, 128], BF16, tag="qtab")
        nc.sync.dma_start(qt[:], dram["qtab"][i * 128:(i + 1) * 128, :])
        qtab_t[i] = qt

    def dma_bias(i):
        bt = biasp.tile([128, 2 * Lk], BF16, tag="bias")
        nc.sync.dma_start(bt[:], dram["bias2"][i * 128:(i + 1) * 128, :])
        bias_t[i] = bt

    def qproj(i):
        ps0 = pmm.tile([128, 512], F32, tag="mm")
        ps1 = pmm.tile([128, 512], F32, tag="mm")
        xt = xt_t[i]
        for kc in range(8):
            mm(ps0[:], xt[:, kc * 128:(kc + 1) * 128], wq[kc][:, 0:512],
               (kc == 0), (kc == 7))
        for kc in range(8):
            mm(ps1[:], xt[:, kc * 128:(kc + 1) * 128], wq[kc][:, 512:1024],
               (kc == 0), (kc == 7))
        psq_t[i] = (ps0, ps1)
        xt_t[i] = None

    def q_square(i):
        # ACT: square both psum halves -> sq bf16
        sq = sqp.tile([128, C], BF16, tag="sq")
        ps0, ps1 = psq_t[i]
        nc.scalar.activation(sq[:, 0:512], ps0[:], AF.Square)
        nc.scalar.activation(sq[:, 512:1024], ps1[:], AF.Square)
        sq_t[i] = sq

    def q_reduce(i):
        s16 = smalls.tile([128, H], F32, tag="s16")
        nc.vector.reduce_sum(s16[:], sq_t[i][:].rearrange("p (h d) -> p h d", d=D),
                             axis=AX)
        sq_t[i] = None
        return s16

    def q_alpha_sqrt(i, s16):
        # rsqrt(s) = exp(-0.5*ln(s)); ln+exp+square+copy share one ACT
        # function-table set, so no ACT_TABLE_LOAD thrash (Sqrt doesn't).
        rt = smalls.tile([128, H], F32, tag="rt")
        nc.scalar.activation(rt[:], s16[:], AF.Ln, bias=eps[:])
        ar = smalls.tile([128, H], F32, tag="ar")
        nc.scalar.activation(ar[:], rt[:], AF.Exp, scale=-0.5)
        return ar

    def q_alpha_fin(i, ar):
        al = smalls.tile([128, H], F32, tag="alpha")
        nc.vector.tensor_mul(al[:], ar[:], sm_r[:])
        alpha_t[i] = al

    def q_rope(i):
        ps0, ps1 = psq_t[i]
        al = alpha_t[i]
        qh = qwork.tile([128, C], BF16, tag="qh")
        nc.vector.tensor_mul(
            qh[:, 0:512].rearrange("p (h d) -> p h d", d=D),
            ps0[:].rearrange("p (h d) -> p h d", d=D),
            al[:, 0:8].unsqueeze(2).to_broadcast((128, 8, D)))
        nc.vector.tensor_mul(
            qh[:, 512:1024].rearrange("p (h d) -> p h d", d=D),
            ps1[:].rearrange("p (h d) -> p h d", d=D),
            al[:, 8:16].unsqueeze(2).to_broadcast((128, 8, D)))
        psq_t[i] = None
        qt = qtab_t[i]
        qa = qwork.tile([128, C], BF16, tag="qa")
        nc.gpsimd.tensor_mul(
            qa[:].rearrange("p (h d) -> p h d", d=D),
            qh[:].rearrange("p (h d) -> p h d", d=D),
            qt[:, 0:64].unsqueeze(1).to_broadcast((128, H, D)))
        qb = qwork.tile([128, C], BF16, tag="qb")
        qh4 = qh[:].rearrange("p (h j t) -> p h j t", j=32, t=2)
        qb4 = qb[:].rearrange("p (h j t) -> p h j t", j=32, t=2)
        nc.gpsimd.tensor_mul(
            qb4[:, :, :, 0:1].squeeze(3),
            qh4[:, :, :, 1:2].squeeze(3),
            qt[:, 64:96].unsqueeze(1).to_broadcast((128, H, 32)))
        nc.gpsimd.tensor_mul(
            qb4[:, :, :, 1:2].squeeze(3),
            qh4[:, :, :, 0:1].squeeze(3),
            qt[:, 96:128].unsqueeze(1).to_broadcast((128, H, 32)))
        qr = qwork.tile([128, C], BF16, tag="qr")
        nc.vector.tensor_add(qr[:], qa[:], qb[:])
        qr_t[i] = qr
        qtab_t[i] = None

    def q_transpose(i):
        ps = ptp.tile([128, C], BF16, tag="tp")
        for ct in range(8):
            tr(ps[:, ct * 128:(ct + 1) * 128], qr_t[i][:, ct * 128:(ct + 1) * 128],
               ident[:])
        return ps

    def q_transpose_copy(i, ps):
        sb = qTp.tile([128, C], BF16, tag="qT")
        nc.vector.tensor_copy(sb[:], ps[:])
        qT_t[i] = sb
        qr_t[i] = None

    def logits(i, pairs):
        if lg_t[i] is None:
            lg_t[i] = [None] * 8
        for p in pairs:
            ps = plg.tile([128, 2 * Lk], F32, tag="lg")
            mm(ps[:], ident[:], bias_t[i][:], True, False)
            mm(ps[:], qT_t[i][:, 128 * p:128 * (p + 1)], kT[p][:], False, True)
            lg_t[i][p] = ps

    def exp_pairs(i, pairs):
        if at_t[i] is None:
            at_t[i] = atp.tile([128, H * Lk], BF16, tag="at", name="at")
        at = at_t[i]
        for p in pairs:
            nc.scalar.activation(at[:, 2 * p * Lk:2 * (p + 1) * Lk],
                                 lg_t[i][p][:], AF.Exp)
            lg_t[i][p] = None

    def softmax_rec(i):
        # denominators: reduce over k per head (Pool can't do free-axis)
        s_all = smalls.tile([128, H], F32, tag="s_all")
        at = at_t[i]
        nc.vector.reduce_sum(
            s_all[:], at[:].rearrange("p (h k) -> p h k", k=Lk), axis=AX)
        s_t[i] = s_all
        rec = smalls.tile([128, H], BF16, tag="rec")
        with nc.allow_low_precision(reason="1/s broadcast tile; bf16 is enough"):
            nc.vector.reciprocal(rec[:], s_t[i][:])
        rec_t[i] = rec
        # gather tails: at[:, h*Lk + 128 : (h+1)*Lk] -> tails [128, (h,16)]
        tails = smalls.tile([128, H * 16], BF16, tag="tails")
        nc.gpsimd.tensor_copy(
            tails[:].rearrange("p (h k) -> p h k", k=16),
            at_t[i][:].rearrange("p (h k) -> p h k", k=Lk)[:, :, 128:Lk])
        tails_t[i] = tails

    def attn_transpose(i):
        # main: 16 heads -> 2 psum tiles of 8 heads; tails -> 1 small psum
        pss = []
        for c in range(2):
            ps = ptp.tile([128, 8 * 128], BF16, tag="tp")
            for hh in range(8):
                h = 8 * c + hh
                tr(ps[:, hh * 128:(hh + 1) * 128],
                   at_t[i][:, h * Lk:h * Lk + 128], ident[:])
            pss.append(ps)
        pstl = ptp.tile([128, 256], BF16, tag="tp")
        for c in range(2):
            tr(pstl[:, c * 128:(c + 1) * 128],
               tails_t[i][:, c * 128:(c + 1) * 128], ident[:])
        # rec transpose: [128,16] -> [16,128]
        psrec = ptp.tile([16, 128], BF16, tag="tp")
        tr(psrec[:], rec_t[i][:], ident[:])
        return pss, pstl, psrec

    def attn_transpose_copy(i, pss, pstl, psrec):
        aTs = []
        for c in range(2):
            sb = aTp.tile([128, 8 * 128], BF16, tag="aT")
            nc.vector.tensor_copy(sb[:], pss[c][:])
            aTs.append(sb)
        aT_t[i] = aTs
        tl = aTp.tile([128, 256], BF16, tag="tailT")
        nc.vector.tensor_copy(tl[:], pstl[:])
        tailT_t[i] = tl
        rT = smalls.tile([16, 128], BF16, tag="recT")
        nc.vector.tensor_copy(rT[:], psrec[:])
        recT_t[i] = rT
        at_t[i] = None
        tails_t[i] = None
        rec_t[i] = None

    def mult_expand(i):
        m0 = pmm.tile([128, 512], F32, tag="mm")
        m1 = pmm.tile([128, 512], F32, tag="mm")
        for p in range(8):
            dst = (m0 if p < 4 else m1)
            mm(dst[:, (p % 4) * 128:(p % 4 + 1) * 128],
               sel_sb[:, 128 * p:128 * (p + 1)], recT_t[i][:], True, True)
        # only one PSUM operand allowed per DVE op -> stage mult in SBUF
        # (GPSIMD can't read PSUM, so use ACT)
        msb = oupp.tile([128, C], BF16, tag="mult")
        nc.scalar.copy(msb[:, 0:512], m0[:])
        nc.scalar.copy(msb[:, 512:1024], m1[:])
        mult_t[i] = msb

    def pv(i):
        o0 = pmm.tile([128, 512], F32, tag="mm")
        o1 = pmm.tile([128, 512], F32, tag="mm")
        aTs, tl = aT_t[i], tailT_t[i]
        for p in range(8):
            dst = (o0 if p < 4 else o1)
            reg = dst[:, (p % 4) * 128:(p % 4 + 1) * 128]
            for j in range(2):
                h = 2 * p + j
                mm(reg[64 * j:64 * j + 64, :],
                   v_m[:, h * D:(h + 1) * D],
                   aTs[h // 8][:, (h % 8) * 128:(h % 8 + 1) * 128],
                   True, False, skip_group_check=True)
            r = 32 * (p % 4)
            mm(reg, vtz[r:r + 32, 128 * p:128 * (p + 1)],
               tl[r:r + 32, 128 * (p // 4):128 * (p // 4) + 128],
               False, True, skip_group_check=True, tile_position=(r, 0))
        pso_t[i] = (o0, o1)

    def divide(i):
        o0, o1 = pso_t[i]
        msb = mult_t[i]
        sb = oupp.tile([128, C], BF16, tag="oupT")
        nc.vector.tensor_mul(sb[:, 0:512], o0[:], msb[:, 0:512])
        nc.vector.tensor_mul(sb[:, 512:1024], o1[:], msb[:, 512:1024])
        oupT_t[i] = sb
        pso_t[i] = None
        mult_t[i] = None
        aT_t[i] = None
        tailT_t[i] = None
        recT_t[i] = None

    def outproj(i):
        ps0 = pmm.tile([128, 512], F32, tag="mm")
        ps1 = pmm.tile([128, 512], F32, tag="mm")
        for p in range(8):
            mm(ps0[:], oupT_t[i][:, 128 * p:128 * (p + 1)], wp[p][:, 0:512],
               (p == 0), (p == 7))
        for p in range(8):
            mm(ps1[:], oupT_t[i][:, 128 * p:128 * (p + 1)], wp[p][:, 512:1024],
               (p == 0), (p == 7))
        return ps0, ps1

    def outcopy_dma(i, ps0, ps1):
        osb = outp.tile([128, C], F32, tag="osb")
        nc.scalar.copy(osb[:, 0:512], ps0[:])
        nc.scalar.copy(osb[:, 512:1024], ps1[:])
        nc.sync.dma_start(out_d[i * 128:(i + 1) * 128, :], osb[:])
        oupT_t[i] = None

    # ---------------- pipelined main loop ----------------
    # iteration i emits: qT+logits(i-1); attnT+mult+PV+divide(i-2);
    #                    Qproj/q-chain(i); outproj+copy(i-3)
    # PE order tuned so pmm's 4-bank ring reuses each bank only after its
    # DVE/ACT consumer has drained it.
    dma_in(0)
    dma_bias(0)
    dma_in(1)
    for i in range(NLT + 3):
        i0, i1, i2, i3 = i, i - 1, i - 2, i - 3
        # prefetch
        if i + 2 < NLT:
            dma_in(i + 2)
        if i + 1 < NLT:
            dma_bias(i + 1)

        # PE stage 1: q transpose (i-1), then its DVE drain
        if 0 <= i1 < NLT:
            psqT = q_transpose(i1)
            q_transpose_copy(i1, psqT)
        # PE stage 2: attn transposes (i-2), then their DVE drains
        if 0 <= i2 < NLT:
            trs = attn_transpose(i2)
            attn_transpose_copy(i2, *trs)
        # PE stage 3a: logits (i-1) pairs 0-3 (+ACT exp)
        if 0 <= i1 < NLT:
            logits(i1, range(0, 4))
            exp_pairs(i1, range(0, 4))
        # PE stage 4+5: mult expand + PV (i-2), then DVE divide
        if 0 <= i2 < NLT:
            mult_expand(i2)
            pv(i2)
            divide(i2)
        # PE stage 6: Q proj (i); ACT square; DVE reduce; ACT sqrt
        if 0 <= i0 < NLT:
            qproj(i0)
            q_square(i0)
            s16 = q_reduce(i0)
            rt = q_alpha_sqrt(i0, s16)
        # PE stage 3b: logits (i-1) pairs 4-7 (+ACT exp, DVE rec)
        if 0 <= i1 < NLT:
            logits(i1, range(4, 8))
            exp_pairs(i1, range(4, 8))
            softmax_rec(i1)
        # PE stage 7: out projection (i-3)
        if 0 <= i3 < NLT:
            ps0, ps1 = outproj(i3)
            outcopy_dma(i3, ps0, ps1)
        # q-chain tail (i): alpha + rope (DVE/Pool)
        if 0 <= i0 < NLT:
            q_alpha_fin(i0, rt)
            q_rope(i0)
    ctx.close()


def run(inputs, trace=False, nc=None):
    in_maps = host_prep(inputs)
    if nc is None:
        nc = build()
    res = run_bass_kernel_spmd(nc, in_maps, core_ids=list(range(8)), trace=trace)
    outs = np.stack([res.results[b]["out"][:L, :] for b in range(B)])
    return outs, res


if __name__ == "__main__":
    import time
    t0 = time.time()
    nc = build()
    print("BUILD OK", time.time() - t0, "s")


_NC_CACHE = {}


def kernel(**inputs):
    """Full unsharded inputs -> full output [8, 1704, 1024] float32.

    Data-parallel over batch: core b computes batch element b on NeuronCore b.
    """
    if "v2" not in _NC_CACHE:
        _NC_CACHE["v2"] = build()
    out, _ = run(inputs, trace=False, nc=_NC_CACHE["v2"])
    return out.astype(np.float32)
